# revision 20
# baseline (speedup 1.0000x reference)
"""Trainium2 Bass kernel for nn_Attention_30760555774660 (stacked attention VQA net).

Sharding: data-parallel over batch, 256 -> 8 cores x 32. Weights replicated.

v3 design (per core: B=32, S=196, D=1024, A=512, O=3000):
  - All big matmuls in bf16 (tolerance gate is 2e-2; bf16 lands ~3e-3).
  - Host supplies img in BOTH layouts: native [b, s-chunks, d] for the
    attention-weighted sums, and pair-packed transposed [pair, d-part, c,
    392] for the projections (two batch elems side by side in the free dim
    so every projection matmul streams N=392).
  - Projections run transposed: projT[a, s2] = W_ia[:, a-chunk].T @ imgTP,
    W chunks stationary, out [128a, 392] PSUM (one bank per a-chunk).
  - q-projection broadcast + b_qa fold into tanh as the per-partition
    activation bias (QP1T/QP2T [a-part, b]).
  - logits = Wp.T @ haT on PE (M=1, N=392 per pair), softmax per b on one
    partition, E transposed back to [s, 1] via tiny PE transposes into a
    pre-masked [s, 8] group tile; vI for 8 b's accumulates into two
    [8, 512] PSUM banks.
  - u = vI*R + prev via one fused scalar_tensor_tensor per 512-chunk.
  - Final FC streams W_fc bf16 tiles (16 prefetched during the loop)
    against stationary u2T columns; b_fc folds in via a K=1 ones matmul.
"""

import sys

import numpy as np

if "/opt/trn_rl_repo" not in sys.path:
    sys.path.insert(0, "/opt/trn_rl_repo")

B_FULL = 256
N_CORES = 8
B = B_FULL // N_CORES  # 32
S = 196
S2 = 2 * S  # 392
D = 1024
A = 512
O = 3000
DC = 8  # d chunks of 128
AC = 4  # a chunks of 128
OB = 8  # batch group (oct)
NOCT = B // OB  # 4
NPAIR = B // 2  # 16
ON = 500
OC = 6
S_CHUNKS = ((0, 128), (1, 68))

_nc_cache = None


def _build_nc():
    import concourse.bacc as bacc
    import concourse.tile as tile
    from concourse import mybir

    f32 = mybir.dt.float32
    bf16 = mybir.dt.bfloat16
    Tanh = mybir.ActivationFunctionType.Tanh
    Exp = mybir.ActivationFunctionType.Exp
    mult = mybir.AluOpType.mult
    add = mybir.AluOpType.add

    nc = bacc.Bacc("TRN2", target_bir_lowering=False)

    imgN_h = nc.dram_tensor("imgN", [B, 128, 2, D], bf16, kind="ExternalInput")
    imgTP_h = nc.dram_tensor("imgTP", [NPAIR, 128, DC, S2], bf16, kind="ExternalInput")
    quesN_h = nc.dram_tensor("quesN", [B, D], f32, kind="ExternalInput")
    wia1_h = nc.dram_tensor("wia1", [128, DC, A], bf16, kind="ExternalInput")
    wia2_h = nc.dram_tensor("wia2", [128, DC, A], bf16, kind="ExternalInput")
    wqa2_h = nc.dram_tensor("wqa2", [128, DC, A], bf16, kind="ExternalInput")
    wfc_h = nc.dram_tensor("wfc", [128, DC, O], bf16, kind="ExternalInput")
    bfc_h = nc.dram_tensor("bfc", [1, O], bf16, kind="ExternalInput")
    # miscb bf16 [128, 48]: col0 = 1.0 (transpose ident), 1:5 wp1, 5:9 wp2,
    # cols 9:41 row-0 ones (FC bias matmul lhsT)
    miscb_h = nc.dram_tensor("miscb", [128, 48], bf16, kind="ExternalInput")
    # miscf f32 [128, 140]: 0:8 eye(8), 8:12 bqa2T, 12:140 host-computed QP1T
    miscf_h = nc.dram_tensor("miscf", [128, 140], f32, kind="ExternalInput")
    score_h = nc.dram_tensor("score", [B, O], f32, kind="ExternalOutput")

    from contextlib import ExitStack

    with tile.TileContext(nc) as tc:
        with ExitStack() as stack:
            pool = lambda **kw: stack.enter_context(tc.tile_pool(**kw))
            const = pool(name="const", bufs=1)
            imgn_p = pool(name="imgn", bufs=9)
            imgt_p = pool(name="imgt", bufs=6)
            hap = pool(name="ha", bufs=8)
            parkp = pool(name="park", bufs=5)
            ep = pool(name="ep", bufs=3)
            etp = pool(name="etp", bufs=3)
            zp = pool(name="zp", bufs=4)
            rp = pool(name="rp", bufs=4)
            qpool = pool(name="qp", bufs=2)
            upool = pool(name="up", bufs=3)
            u1tp = pool(name="u1tp", bufs=2)
            qp2tp = pool(name="qp2tp", bufs=2)
            wfp = pool(name="wf", bufs=37)
            scp = pool(name="sc", bufs=2)
            pps = pool(name="psproj", bufs=3, space="PSUM")
            pvi = pool(name="psvi", bufs=2, space="PSUM")
            psf = pool(name="pssmf", bufs=3, space="PSUM")
            wf_pre = []

            # ---- early constants (needed by the first projections) ----
            miscb = const.tile([128, 48], bf16, tag="miscb")
            nc.sync.dma_start(out=miscb, in_=miscb_h[:, :])
            miscf = const.tile([128, 140], f32, tag="miscf")
            nc.sync.dma_start(out=miscf, in_=miscf_h[:, :])
            wia1 = const.tile([128, DC, A], bf16, tag="wia1")
            nc.sync.dma_start(out=wia1, in_=wia1_h[:, :, :])
            identb = miscb
            identf = miscf
            wp1 = miscb[:, 1 : 1 + AC]
            wp2 = miscb[:, 5 : 5 + AC]
            onesb = miscb[:, 9 : 9 + B]
            bqa2T = miscf[:, 8 : 8 + AC]
            QP1Tap = lambda a, col: miscf[:, 12 + a * B + col : 13 + a * B + col]

            def load_oct_dmas(g):
                g8 = g * OB
                itps = []
                for pp_ in range(OB // 2):
                    pr = g * (OB // 2) + pp_
                    itb = imgt_p.tile([128, DC, S2], bf16, tag="imgt", name=f"itp_{pr}")
                    nc.sync.dma_start(
                        out=itb,
                        in_=imgTP_h[pr : pr + 1, :, :, :].rearrange(
                            "o p c s -> (o p) c s"
                        ),
                    )
                    itps.append(itb)
                q8 = qpool.tile([OB, D], f32, tag="q8", name=f"q8_{g}")
                nc.sync.dma_start(out=q8, in_=quesN_h[g8 : g8 + OB, :])
                inbs = []
                for bb in range(OB):
                    b = g8 + bb
                    inb = imgn_p.tile([128, 2, D], bf16, tag="imgn", name=f"inb_{b}")
                    nc.gpsimd.dma_start(
                        out=inb,
                        in_=imgN_h[b : b + 1, :, :, :].rearrange("o p k d -> (o p) k d"),
                    )
                    inbs.append(inb)
                return q8, inbs, itps

            q8_g, inbs_g, itps_g = {}, {}, {}
            q8_g[0], inbs_g[0], itps_g[0] = load_oct_dmas(0)

            # ---- remaining constants ----
            wia2 = const.tile([128, DC, A], bf16, tag="wia2")
            nc.sync.dma_start(out=wia2, in_=wia2_h[:, :, :])
            wqa2 = const.tile([128, DC, A], bf16, tag="wqa2")
            nc.sync.dma_start(out=wqa2, in_=wqa2_h[:, :, :])
            bfc = const.tile([1, O], bf16, tag="bfc")
            nc.sync.dma_start(out=bfc, in_=bfc_h[:, :])
            u2T = const.tile([128, DC, B], bf16, tag="u2T")

            def proj_tanh(itb, wia, QPT, bias_cols, out_ha, parked):
                """Pair projection + tanh (or park copy) per a-chunk.

                out_ha: [128, AC, S2] bf16 target; bias_cols: (col0, col1) into
                QPT for the two batch elems, or None to park (plain copy)."""
                for a in range(AC):
                    ppt = pps.tile([128, S2], f32, tag="proj", name=f"pj_{id(out_ha)}_{a}")
                    for d in range(DC):
                        nc.tensor.matmul(
                            ppt,
                            wia[:, d, a * 128 : (a + 1) * 128],
                            itb[:, d, :],
                            start=(d == 0),
                            stop=(d == DC - 1),
                        )
                    if parked:
                        if a % 2 == 0:
                            nc.scalar.copy(out_ha[:, a, :], ppt)
                        else:
                            nc.vector.tensor_copy(out_ha[:, a, :], ppt)
                    else:
                        for h in range(2):
                            nc.scalar.activation(
                                out_ha[:, a, h * S : (h + 1) * S],
                                ppt[:, h * S : (h + 1) * S],
                                Tanh,
                                bias=QPT(a, bias_cols[h]),
                            )

            def tanh_parked(pk, QPT, bias_cols, out_ha):
                for a in range(AC):
                    for h in range(2):
                        nc.scalar.activation(
                            out_ha[:, a, h * S : (h + 1) * S],
                            pk[:, a, h * S : (h + 1) * S],
                            Tanh,
                            bias=QPT(a, bias_cols[h]),
                        )

            def logits_softmax(ha, wp, et, z, bb0):
                """Pair logits -> per-b exp -> E^T columns into group tile."""
                lg = psf.tile([1, S2], f32, tag="smf", name=f"lg_{id(ha)}")
                for c in range(AC):
                    nc.tensor.matmul(
                        lg, wp[:, c : c + 1], ha[:, c, :], start=(c == 0), stop=(c == AC - 1)
                    )
                for h in range(2):
                    bb = bb0 + h
                    E = ep.tile([1, S], bf16, tag="E", name=f"E_{id(ha)}_{h}")
                    nc.scalar.activation(
                        E, lg[0:1, h * S : (h + 1) * S], Exp,
                        accum_out=z[0:1, bb : bb + 1],
                    )
                    for si, sl in S_CHUNKS:
                        pt = psf.tile([128, 1], bf16, tag="smf", name=f"pt_{id(ha)}_{h}_{si}")
                        nc.tensor.transpose(
                            pt[0:sl, :], E[0:1, si * 128 : si * 128 + sl],
                            identb[0:1, 0:1],
                        )
                        nc.vector.tensor_copy(et[0:sl, si, bb, bb : bb + 1], pt[0:sl, :])

            def group_vI_u(et, z, inbs, other, nm):
                """vI for 8 b's + fused u = vI*R + other. Returns u [8,1024] f32."""
                ztp = psf.tile([OB, 1], f32, tag="smf", name=f"ztp_{nm}")
                nc.tensor.transpose(ztp[0:OB, :], z[0:1, 0:OB], identf[0:1, 0:1])
                R = rp.tile([OB, 1], f32, tag="R", name=f"R_{nm}")
                nc.vector.reciprocal(R, ztp[0:OB, :])
                vis = [
                    pvi.tile([OB, A], f32, tag="vi", name=f"vi_{nm}_0"),
                    pvi.tile([OB, A], f32, tag="vi", name=f"vi_{nm}_1"),
                ]
                for n in range(2):
                    for bb in range(OB):
                        for si, sl in S_CHUNKS:
                            nc.tensor.matmul(
                                vis[n],
                                et[0:sl, si, bb, :],
                                inbs[bb][0:sl, si, n * A : (n + 1) * A],
                                start=(bb == 0 and si == 0),
                                stop=(bb == OB - 1 and si == 1),
                            )
                u = upool.tile([OB, D], f32, tag="u", name=f"u_{nm}")
                for n in range(2):
                    nc.vector.scalar_tensor_tensor(
                        u[0:OB, n * A : (n + 1) * A],
                        vis[n],
                        R[0:OB, 0:1],
                        other[0:OB, n * A : (n + 1) * A],
                        op0=mult,
                        op1=add,
                    )
                return u

            def transpose_cols(u, dst_ap_fn, nm):
                """u [8, 1024] f32 -> 8 chunk transposes into dst [128, c, 8]."""
                for c in range(DC):
                    pt = psf.tile([128, OB], f32, tag="smf", name=f"ut_{nm}_{c}")
                    nc.tensor.transpose(
                        pt[:, 0:OB], u[0:OB, c * 128 : (c + 1) * 128], identf[0:OB, 0:OB]
                    )
                    nc.vector.tensor_copy(dst_ap_fn(c), pt[:, 0:OB])

            # ------- software-pipelined phases: A=blk1, B=proj2 parks, -------
            # ------- C=u1/QP2T, D=tanh2/logits2, E=u2; A(g+1) fills D/E(g) ----
            st = {}

            def emit_wf(cnt):
                for _ in range(cnt):
                    k = len(wf_pre)
                    wf = wfp.tile([128, ON], bf16, tag="wf", name=f"wf_{k}")
                    n, c = divmod(k, DC)
                    nc.sync.dma_start(out=wf, in_=wfc_h[:, c, n * ON : (n + 1) * ON])
                    wf_pre.append(wf)

            def phase_A(g):
                g8 = g * OB
                itps = itps_g[g]
                et1 = etp.tile([128, 2, OB, OB], bf16, tag="et", name=f"et1_{g}")
                nc.vector.memset(et1, 0.0)
                z1 = zp.tile([1, OB], f32, tag="z", name=f"z1_{g}")
                for pp_ in range(OB // 2):
                    ha = hap.tile([128, AC, S2], bf16, tag="ha", name=f"ha1_{g}_{pp_}")
                    proj_tanh(
                        itps[pp_], wia1, QP1Tap,
                        (g8 + 2 * pp_, g8 + 2 * pp_ + 1), ha, parked=False,
                    )
                    logits_softmax(ha, wp1, et1, z1, 2 * pp_)
                st[g] = {"et1": et1, "z1": z1}

            def phase_B(g):
                itps = itps_g[g]
                parks = []
                for pp_ in range(OB // 2):
                    pk = parkp.tile(
                        [128, AC, S2], bf16, tag="park", name=f"pk_{g}_{pp_}"
                    )
                    proj_tanh(itps[pp_], wia2, None, None, pk, parked=True)
                    parks.append(pk)
                st[g]["parks"] = parks

            def phase_C(g):
                s = st[g]
                u1 = group_vI_u(s["et1"], s["z1"], inbs_g[g], q8_g[g], f"u1_{g}")
                u1T = u1tp.tile([128, DC, OB], bf16, tag="u1T", name=f"u1T_{g}")
                transpose_cols(u1, lambda c: u1T[:, c, :], f"u1_{g}")
                QP2T = qp2tp.tile([128, AC, OB], f32, tag="qp2T", name=f"QP2T_{g}")
                for a in range(AC):
                    pc2 = psf.tile([128, OB], f32, tag="smf", name=f"qp2_{g}_{a}")
                    for d in range(DC):
                        nc.tensor.matmul(
                            pc2, wqa2[:, d, a * 128 : (a + 1) * 128], u1T[:, d, :],
                            start=(d == 0), stop=(d == DC - 1),
                        )
                    nc.vector.tensor_scalar_add(
                        QP2T[:, a, :], pc2, bqa2T[:, a : a + 1]
                    )
                s["u1"] = u1
                s["QP2T"] = QP2T

            def phase_D(g):
                s = st[g]
                QP2T = s["QP2T"]
                et2 = etp.tile([128, 2, OB, OB], bf16, tag="et", name=f"et2_{g}")
                nc.vector.memset(et2, 0.0)
                z2 = zp.tile([1, OB], f32, tag="z", name=f"z2_{g}")
                for pp_ in range(OB // 2):
                    ha2 = hap.tile([128, AC, S2], bf16, tag="ha", name=f"ha2_{g}_{pp_}")
                    tanh_parked(
                        s["parks"][pp_],
                        lambda a, col: QP2T[:, a, col : col + 1],
                        (2 * pp_, 2 * pp_ + 1), ha2,
                    )
                    logits_softmax(ha2, wp2, et2, z2, 2 * pp_)
                s["et2"] = et2
                s["z2"] = z2

            def phase_E(g):
                g8 = g * OB
                s = st[g]
                u2 = group_vI_u(s["et2"], s["z2"], inbs_g[g], s["u1"], f"u2_{g}")
                transpose_cols(u2, lambda c: u2T[:, c, g8 : g8 + OB], f"u2_{g}")

            def load(g):
                q8_g[g], inbs_g[g], itps_g[g] = load_oct_dmas(g)

            phase_A(0)
            load(1)
            phase_B(0)
            phase_C(0)
            emit_wf(8)
            for g in range(NOCT - 1):
                phase_A(g + 1)
                if g + 2 < NOCT:
                    load(g + 2)
                phase_D(g)
                phase_E(g)
                emit_wf(12 if g < 2 else 6)
                phase_B(g + 1)
                phase_C(g + 1)
            phase_D(NOCT - 1)
            phase_E(NOCT - 1)
            emit_wf(2)

            # ---------------- final FC ----------------
            for n in range(OC):
                wfn = []
                for c in range(DC):
                    k = n * DC + c
                    if k < len(wf_pre):
                        wfn.append(wf_pre[k])
                    else:
                        wf = wfp.tile([128, ON], bf16, tag="wf", name=f"wfl_{n}_{c}")
                        nc.sync.dma_start(out=wf, in_=wfc_h[:, c, n * ON : (n + 1) * ON])
                        wfn.append(wf)
                sp = pps.tile([B, ON], f32, tag="proj", name=f"sp_{n}")
                for c in range(DC):
                    nc.tensor.matmul(sp, u2T[:, c, :], wfn[c], start=(c == 0), stop=False)
                nc.tensor.matmul(
                    sp, onesb[0:1, :], bfc[0:1, n * ON : (n + 1) * ON],
                    start=False, stop=True,
                )
                sc = scp.tile([B, ON], f32, tag="sc", name=f"sc_{n}")
                nc.scalar.copy(sc, sp)
                nc.sync.dma_start(out=score_h[:, n * ON : (n + 1) * ON], in_=sc)

    nc.compile()
    return nc


def _get_nc():
    global _nc_cache
    if _nc_cache is None:
        _nc_cache = _build_nc()
    return _nc_cache


def _make_in_maps(inputs):
    import ml_dtypes

    bf = ml_dtypes.bfloat16

    def f32a(x):
        return np.ascontiguousarray(np.asarray(x), np.float32)

    def wchunk(w):  # [D, N] -> [128, DC, N]
        w = f32a(w)
        return np.ascontiguousarray(
            w.reshape(DC, 128, w.shape[1]).transpose(1, 0, 2).astype(bf)
        )

    def acolT(v, dt):  # [A] -> [128, AC]
        return np.ascontiguousarray(f32a(v).reshape(AC, 128).T.astype(dt))

    img = f32a(inputs["img_feat"])  # [256, 196, 1024]
    ques = f32a(inputs["ques_feat"])  # [256, 1024]

    imgN = np.zeros((B_FULL, 128, 2, D), np.float32)
    imgN[:, :, 0, :] = img[:, 0:128, :]
    imgN[:, 0:68, 1, :] = img[:, 128:196, :]
    imgN = np.ascontiguousarray(imgN.astype(bf))
    # [B, 128, DC, S] then pack pairs along the last axis -> [B//2, 128, DC, 392]
    imgT = img.reshape(B_FULL, S, DC, 128).transpose(0, 3, 2, 1)
    imgTP = np.concatenate(
        [imgT[0::2], imgT[1::2]], axis=3
    )  # [128 pairs, 128, DC, 392]
    imgTP = np.ascontiguousarray(imgTP.astype(bf))

    miscb = np.zeros((128, 48), np.float32)
    miscb[0, 0] = 1.0
    miscb[:, 1 : 1 + AC] = f32a(inputs["Wp1"]).reshape(AC, 128).T
    miscb[:, 5 : 5 + AC] = f32a(inputs["Wp2"]).reshape(AC, 128).T
    miscb[0, 9 : 9 + B] = 1.0
    miscb = np.ascontiguousarray(miscb.astype(bf))

    # QP1 = ques @ W_qa1 + b_qa1, computed on host in fp32
    qp1_full = ques @ f32a(inputs["W_qa1"]) + f32a(inputs["b_qa1"])  # [256, 512]

    shared = {
        "wia1": wchunk(inputs["W_ia1"]),
        "wia2": wchunk(inputs["W_ia2"]),
        "wqa2": wchunk(inputs["W_qa2"]),
        "wfc": wchunk(inputs["W_fc"]),
        "bfc": np.ascontiguousarray(f32a(inputs["b_fc"]).reshape(1, O).astype(bf)),
        "miscb": miscb,
    }
    bqa2T = acolT(inputs["b_qa2"], np.float32)
    in_maps = []
    for core in range(N_CORES):
        sl = slice(core * B, (core + 1) * B)
        slp = slice(core * NPAIR, (core + 1) * NPAIR)
        m = dict(shared)
        m["imgN"] = imgN[sl]
        m["imgTP"] = imgTP[slp]
        m["quesN"] = np.ascontiguousarray(ques[sl])
        miscf = np.zeros((128, 140), np.float32)
        miscf[0:8, 0:8] = np.eye(8)
        miscf[:, 8 : 8 + AC] = bqa2T
        miscf[:, 12 : 12 + AC * B] = (
            qp1_full[sl].reshape(B, AC, 128).transpose(2, 1, 0).reshape(128, AC * B)
        )
        m["miscf"] = np.ascontiguousarray(miscf)
        in_maps.append(m)
    return in_maps


def kernel_run(inputs, trace=False):
    from concourse.bass_utils import run_bass_kernel_spmd

    nc = _get_nc()
    in_maps = _make_in_maps(inputs)
    res = run_bass_kernel_spmd(nc, in_maps, core_ids=list(range(N_CORES)), trace=trace)
    out = np.concatenate([r["score"] for r in res.results], axis=0)
    return out, res


def kernel(**inputs):
    out, _ = kernel_run(inputs)
    return out


# revision 22
# speedup vs baseline: 1.5234x; 1.5234x over previous
"""Trainium2 Bass kernel for nn_Attention_30760555774660 (stacked attention VQA net).

Sharding: data-parallel over batch, 256 -> 8 cores x 32. Weights replicated.

v3 design (per core: B=32, S=196, D=1024, A=512, O=3000):
  - All big matmuls in bf16 (tolerance gate is 2e-2; bf16 lands ~3e-3).
  - Host supplies img in BOTH layouts: native [b, s-chunks, d] for the
    attention-weighted sums, and pair-packed transposed [pair, d-part, c,
    392] for the projections (two batch elems side by side in the free dim
    so every projection matmul streams N=392).
  - Projections run transposed: projT[a, s2] = W_ia[:, a-chunk].T @ imgTP,
    W chunks stationary, out [128a, 392] PSUM (one bank per a-chunk).
  - q-projection broadcast + b_qa fold into tanh as the per-partition
    activation bias (QP1T/QP2T [a-part, b]).
  - logits = Wp.T @ haT on PE (M=1, N=392 per pair), softmax per b on one
    partition, E transposed back to [s, 1] via tiny PE transposes into a
    pre-masked [s, 8] group tile; vI for 8 b's accumulates into two
    [8, 512] PSUM banks.
  - u = vI*R + prev via one fused scalar_tensor_tensor per 512-chunk.
  - Final FC streams W_fc bf16 tiles (16 prefetched during the loop)
    against stationary u2T columns; b_fc folds in via a K=1 ones matmul.
"""

import sys

import numpy as np

if "/opt/trn_rl_repo" not in sys.path:
    sys.path.insert(0, "/opt/trn_rl_repo")

B_FULL = 256
N_CORES = 8
B = B_FULL // N_CORES  # 32
S = 196
S2 = 2 * S  # 392
D = 1024
A = 512
O = 3000
DC = 8  # d chunks of 128
AC = 4  # a chunks of 128
OB = 8  # batch group (oct)
NOCT = B // OB  # 4
NPAIR = B // 2  # 16
ON = 500
OC = 6
S_CHUNKS = ((0, 128), (1, 68))

_nc_cache = None


def _build_nc():
    import concourse.bacc as bacc
    import concourse.tile as tile
    from concourse import mybir

    f32 = mybir.dt.float32
    bf16 = mybir.dt.bfloat16
    f8 = mybir.dt.float8e4
    DR = mybir.MatmulPerfMode.DoubleRow
    Tanh = mybir.ActivationFunctionType.Tanh
    Exp = mybir.ActivationFunctionType.Exp
    mult = mybir.AluOpType.mult
    add = mybir.AluOpType.add

    nc = bacc.Bacc("TRN2", target_bir_lowering=False)

    imgN_h = nc.dram_tensor("imgN", [B, 128, 2, D], bf16, kind="ExternalInput")
    imgTP_h = nc.dram_tensor("imgTP", [NPAIR, 128, DC, S2], f8, kind="ExternalInput")
    quesN_h = nc.dram_tensor("quesN", [B, D], f32, kind="ExternalInput")
    wia1_h = nc.dram_tensor("wia1", [128, DC, A], f8, kind="ExternalInput")
    wia2_h = nc.dram_tensor("wia2", [128, DC, A], f8, kind="ExternalInput")
    wqa2_h = nc.dram_tensor("wqa2", [128, DC, A], bf16, kind="ExternalInput")
    wfc_h = nc.dram_tensor("wfc", [128, DC, O], bf16, kind="ExternalInput")
    bfc_h = nc.dram_tensor("bfc", [1, O], bf16, kind="ExternalInput")
    # miscb bf16 [128, 48]: col0 = 1.0 (transpose ident), 1:5 wp1, 5:9 wp2,
    # cols 9:41 row-0 ones (FC bias matmul lhsT)
    miscb_h = nc.dram_tensor("miscb", [128, 48], bf16, kind="ExternalInput")
    # miscf f32 [128, 140]: 0:8 eye(8), 8:12 bqa2T, 12:140 host-computed QP1T
    miscf_h = nc.dram_tensor("miscf", [128, 140], f32, kind="ExternalInput")
    score_h = nc.dram_tensor("score", [B, O], f32, kind="ExternalOutput")

    from contextlib import ExitStack

    with tile.TileContext(nc) as tc:
        with ExitStack() as stack:
            pool = lambda **kw: stack.enter_context(tc.tile_pool(**kw))
            const = pool(name="const", bufs=1)
            imgn_p = pool(name="imgn", bufs=11)
            imgt_p = pool(name="imgt", bufs=6)
            hap = pool(name="ha", bufs=4)
            parkp = pool(name="park", bufs=5)
            ep = pool(name="ep", bufs=4)
            etp = pool(name="etp", bufs=3)
            zp = pool(name="zp", bufs=4)
            rp = pool(name="rp", bufs=4)
            qpool = pool(name="qp", bufs=2)
            upool = pool(name="up", bufs=3)
            u1tp = pool(name="u1tp", bufs=2)
            qp2tp = pool(name="qp2tp", bufs=2)
            wfp = pool(name="wf", bufs=40)
            scp = pool(name="sc", bufs=2)
            pps = pool(name="psproj", bufs=3, space="PSUM")
            pvi = pool(name="psvi", bufs=2, space="PSUM")
            psf = pool(name="pssmf", bufs=3, space="PSUM")
            wf_pre = []

            # ---- early constants (needed by the first projections) ----
            miscb = const.tile([128, 48], bf16, tag="miscb")
            nc.sync.dma_start(out=miscb, in_=miscb_h[:, :])
            miscf = const.tile([128, 140], f32, tag="miscf")
            nc.sync.dma_start(out=miscf, in_=miscf_h[:, :])
            wia1 = const.tile([128, DC, A], f8, tag="wia1")
            nc.sync.dma_start(out=wia1, in_=wia1_h[:, :, :])
            identb = miscb
            identf = miscf
            wp1 = miscb[:, 1 : 1 + AC]
            wp2 = miscb[:, 5 : 5 + AC]
            onesb = miscb[:, 9 : 9 + B]
            bqa2T = miscf[:, 8 : 8 + AC]
            QP1Tap = lambda a, col: miscf[:, 12 + a * B + col : 13 + a * B + col]

            def load_oct_dmas(g):
                g8 = g * OB
                itps = []
                for pp_ in range(OB // 2):
                    pr = g * (OB // 2) + pp_
                    itb = imgt_p.tile([128, DC, S2], f8, tag="imgt", name=f"itp_{pr}")
                    nc.sync.dma_start(
                        out=itb,
                        in_=imgTP_h[pr : pr + 1, :, :, :].rearrange(
                            "o p c s -> (o p) c s"
                        ),
                    )
                    itps.append(itb)
                q8 = qpool.tile([OB, D], f32, tag="q8", name=f"q8_{g}")
                nc.sync.dma_start(out=q8, in_=quesN_h[g8 : g8 + OB, :])
                inbs = []
                for bb in range(OB):
                    b = g8 + bb
                    inb = imgn_p.tile([128, 2, D], bf16, tag="imgn", name=f"inb_{b}")
                    nc.gpsimd.dma_start(
                        out=inb,
                        in_=imgN_h[b : b + 1, :, :, :].rearrange("o p k d -> (o p) k d"),
                    )
                    inbs.append(inb)
                return q8, inbs, itps

            q8_g, inbs_g, itps_g = {}, {}, {}
            q8_g[0], inbs_g[0], itps_g[0] = load_oct_dmas(0)

            # ---- remaining constants ----
            wia2 = const.tile([128, DC, A], f8, tag="wia2")
            nc.sync.dma_start(out=wia2, in_=wia2_h[:, :, :])
            wqa2 = const.tile([128, DC, A], bf16, tag="wqa2")
            nc.sync.dma_start(out=wqa2, in_=wqa2_h[:, :, :])
            bfc = const.tile([1, O], bf16, tag="bfc")
            nc.sync.dma_start(out=bfc, in_=bfc_h[:, :])
            u2T = const.tile([128, DC, B], bf16, tag="u2T")

            def proj_tanh(itb, wia, QPT, bias_cols, out_ha, parked):
                """Pair projection + tanh (or park copy) per a-chunk.

                out_ha: [128, AC, S2] bf16 target; bias_cols: (col0, col1) into
                QPT for the two batch elems, or None to park (plain copy)."""
                for a in range(AC):
                    ppt = pps.tile([128, S2], f32, tag="proj", name=f"pj_{id(out_ha)}_{a}")
                    for d in range(0, DC, 2):
                        nc.tensor.matmul(
                            ppt,
                            wia[:, d : d + 2, a * 128 : (a + 1) * 128],
                            itb[:, d : d + 2, :],
                            start=(d == 0),
                            stop=(d == DC - 2),
                            perf_mode=DR,
                        )
                    if parked:
                        if a % 2 == 0:
                            nc.scalar.copy(out_ha[:, a, :], ppt)
                        else:
                            nc.vector.tensor_copy(out_ha[:, a, :], ppt)
                    else:
                        for h in range(2):
                            nc.scalar.activation(
                                out_ha[:, a, h * S : (h + 1) * S],
                                ppt[:, h * S : (h + 1) * S],
                                Tanh,
                                bias=QPT(a, bias_cols[h]),
                            )

            def tanh_parked(pk, QPT, bias_cols, out_ha):
                for a in range(AC):
                    for h in range(2):
                        nc.scalar.activation(
                            out_ha[:, a, h * S : (h + 1) * S],
                            pk[:, a, h * S : (h + 1) * S],
                            Tanh,
                            bias=QPT(a, bias_cols[h]),
                        )

            def logits_softmax(ha, wp, et, z, bb0):
                """Pair logits -> per-b exp -> E^T columns into group tile."""
                lg = psf.tile([1, S2], f32, tag="smf", name=f"lg_{id(ha)}")
                for c in range(AC):
                    nc.tensor.matmul(
                        lg, wp[:, c : c + 1], ha[:, c, :], start=(c == 0), stop=(c == AC - 1)
                    )
                for h in range(2):
                    bb = bb0 + h
                    E = ep.tile([1, S], bf16, tag="E", name=f"E_{id(ha)}_{h}")
                    nc.scalar.activation(
                        E, lg[0:1, h * S : (h + 1) * S], Exp,
                        accum_out=z[0:1, bb : bb + 1],
                    )
                    for si, sl in S_CHUNKS:
                        pt = psf.tile([128, 1], bf16, tag="smf", name=f"pt_{id(ha)}_{h}_{si}")
                        nc.tensor.transpose(
                            pt[0:sl, :], E[0:1, si * 128 : si * 128 + sl],
                            identb[0:1, 0:1],
                        )
                        nc.vector.tensor_copy(et[0:sl, si, bb, bb : bb + 1], pt[0:sl, :])

            def group_vI_u(et, z, inbs, other, nm):
                """vI for 8 b's + fused u = vI*R + other. Returns u [8,1024] f32."""
                ztp = psf.tile([OB, 1], f32, tag="smf", name=f"ztp_{nm}")
                nc.tensor.transpose(ztp[0:OB, :], z[0:1, 0:OB], identf[0:1, 0:1])
                R = rp.tile([OB, 1], f32, tag="R", name=f"R_{nm}")
                nc.vector.reciprocal(R, ztp[0:OB, :])
                vis = [
                    pvi.tile([OB, A], f32, tag="vi", name=f"vi_{nm}_0"),
                    pvi.tile([OB, A], f32, tag="vi", name=f"vi_{nm}_1"),
                ]
                for n in range(2):
                    for bb in range(OB):
                        for si, sl in S_CHUNKS:
                            nc.tensor.matmul(
                                vis[n],
                                et[0:sl, si, bb, :],
                                inbs[bb][0:sl, si, n * A : (n + 1) * A],
                                start=(bb == 0 and si == 0),
                                stop=(bb == OB - 1 and si == 1),
                            )
                u = upool.tile([OB, D], f32, tag="u", name=f"u_{nm}")
                for n in range(2):
                    nc.vector.scalar_tensor_tensor(
                        u[0:OB, n * A : (n + 1) * A],
                        vis[n],
                        R[0:OB, 0:1],
                        other[0:OB, n * A : (n + 1) * A],
                        op0=mult,
                        op1=add,
                    )
                return u

            def transpose_cols(u, dst_ap_fn, nm):
                """u [8, 1024] f32 -> 8 chunk transposes into dst [128, c, 8]."""
                for c in range(DC):
                    pt = psf.tile([128, OB], f32, tag="smf", name=f"ut_{nm}_{c}")
                    nc.tensor.transpose(
                        pt[:, 0:OB], u[0:OB, c * 128 : (c + 1) * 128], identf[0:OB, 0:OB]
                    )
                    nc.vector.tensor_copy(dst_ap_fn(c), pt[:, 0:OB])

            # ---------------- main loop over octs ----------------
            for g in range(NOCT):
                g8 = g * OB
                if g not in q8_g:
                    q8_g[g], inbs_g[g], itps_g[g] = load_oct_dmas(g)
                q8, inbs, itps = q8_g[g], inbs_g[g], itps_g[g]
                # prefetch next oct's DMAs early
                if g + 1 < NOCT:
                    q8_g[g + 1], inbs_g[g + 1], itps_g[g + 1] = load_oct_dmas(g + 1)
                # prefetch W_fc tiles across the octs (full FC prefetch)
                nwf = 8 if g == 0 else (12 if g < 3 else 8)
                for j in range(nwf):
                    k = len(wf_pre)
                    wf = wfp.tile([128, ON], bf16, tag="wf", name=f"wf_{g}_{j}")
                    n, c = divmod(k, DC)
                    nc.sync.dma_start(out=wf, in_=wfc_h[:, c, n * ON : (n + 1) * ON])
                    wf_pre.append(wf)

                # block 1 per-pair: proj -> tanh -> logits -> exp -> E^T
                et1 = etp.tile([128, 2, OB, OB], bf16, tag="et", name=f"et1_{g}")
                nc.vector.memset(et1, 0.0)
                z1 = zp.tile([1, OB], f32, tag="z", name=f"z1_{g}")
                for pp_ in range(OB // 2):
                    ha = hap.tile([128, AC, S2], bf16, tag="ha", name=f"ha1_{g}_{pp_}")
                    proj_tanh(
                        itps[pp_], wia1, QP1Tap,
                        (g8 + 2 * pp_, g8 + 2 * pp_ + 1), ha, parked=False,
                    )
                    logits_softmax(ha, wp1, et1, z1, 2 * pp_)

                # u1 = vI1 + ques
                u1 = group_vI_u(et1, z1, inbs, q8, f"u1_{g}")

                # u1T (bf16) for the QP2 matvec
                u1T = u1tp.tile([128, DC, OB], bf16, tag="u1T", name=f"u1T_{g}")
                transpose_cols(u1, lambda c: u1T[:, c, :], f"u1_{g}")

                # block 2 projections parked to SBUF (independent of u1);
                # emitted here so their dense matmuls fill the tail
                parks = []
                for pp_ in range(OB // 2):
                    pk = parkp.tile([128, AC, S2], bf16, tag="park", name=f"pk_{g}_{pp_}")
                    proj_tanh(itps[pp_], wia2, None, None, pk, parked=True)
                    parks.append(pk)

                # QP2T = (u1 @ W_qa2 + b_qa2)^T, computed transposed
                QP2T = qp2tp.tile([128, AC, OB], f32, tag="qp2T", name=f"QP2T_{g}")
                for a in range(AC):
                    pc2 = psf.tile([128, OB], f32, tag="smf", name=f"qp2_{g}_{a}")
                    for d in range(DC):
                        nc.tensor.matmul(
                            pc2, wqa2[:, d, a * 128 : (a + 1) * 128], u1T[:, d, :],
                            start=(d == 0), stop=(d == DC - 1),
                        )
                    nc.vector.tensor_scalar_add(
                        QP2T[:, a, :], pc2, bqa2T[:, a : a + 1]
                    )

                # block 2 per-pair tail
                et2 = etp.tile([128, 2, OB, OB], bf16, tag="et", name=f"et2_{g}")
                nc.vector.memset(et2, 0.0)
                z2 = zp.tile([1, OB], f32, tag="z", name=f"z2_{g}")
                for pp_ in range(OB // 2):
                    ha2 = hap.tile([128, AC, S2], bf16, tag="ha", name=f"ha2_{g}_{pp_}")
                    tanh_parked(parks[pp_], lambda a, col: QP2T[:, a, col : col + 1], (2 * pp_, 2 * pp_ + 1), ha2)
                    logits_softmax(ha2, wp2, et2, z2, 2 * pp_)

                # u2 = vI2 + u1
                u2 = group_vI_u(et2, z2, inbs, u1, f"u2_{g}")
                transpose_cols(u2, lambda c: u2T[:, c, g8 : g8 + OB], f"u2_{g}")

            # ---------------- final FC ----------------
            for n in range(OC):
                wfn = []
                for c in range(DC):
                    k = n * DC + c
                    if k < len(wf_pre):
                        wfn.append(wf_pre[k])
                    else:
                        wf = wfp.tile([128, ON], bf16, tag="wf", name=f"wfl_{n}_{c}")
                        nc.sync.dma_start(out=wf, in_=wfc_h[:, c, n * ON : (n + 1) * ON])
                        wfn.append(wf)
                sp = pps.tile([B, ON], f32, tag="proj", name=f"sp_{n}")
                for c in range(DC):
                    nc.tensor.matmul(sp, u2T[:, c, :], wfn[c], start=(c == 0), stop=False)
                nc.tensor.matmul(
                    sp, onesb[0:1, :], bfc[0:1, n * ON : (n + 1) * ON],
                    start=False, stop=True,
                )
                sc = scp.tile([B, ON], f32, tag="sc", name=f"sc_{n}")
                nc.scalar.copy(sc, sp)
                nc.sync.dma_start(out=score_h[:, n * ON : (n + 1) * ON], in_=sc)

    nc.compile()
    return nc


def _get_nc():
    global _nc_cache
    if _nc_cache is None:
        _nc_cache = _build_nc()
    return _nc_cache


def _make_in_maps(inputs):
    import ml_dtypes

    bf = ml_dtypes.bfloat16
    f8 = ml_dtypes.float8_e4m3

    def f32a(x):
        return np.ascontiguousarray(np.asarray(x), np.float32)

    def wchunk(w, dt=None):  # [D, N] -> [128, DC, N]
        w = f32a(w)
        return np.ascontiguousarray(
            w.reshape(DC, 128, w.shape[1]).transpose(1, 0, 2).astype(dt or bf)
        )

    def acolT(v, dt):  # [A] -> [128, AC]
        return np.ascontiguousarray(f32a(v).reshape(AC, 128).T.astype(dt))

    img = f32a(inputs["img_feat"])  # [256, 196, 1024]
    ques = f32a(inputs["ques_feat"])  # [256, 1024]

    imgN = np.zeros((B_FULL, 128, 2, D), np.float32)
    imgN[:, :, 0, :] = img[:, 0:128, :]
    imgN[:, 0:68, 1, :] = img[:, 128:196, :]
    imgN = np.ascontiguousarray(imgN.astype(bf))
    # [B, 128, DC, S] then pack pairs along the last axis -> [B//2, 128, DC, 392]
    imgT = img.reshape(B_FULL, S, DC, 128).transpose(0, 3, 2, 1)
    imgTP = np.concatenate(
        [imgT[0::2], imgT[1::2]], axis=3
    )  # [128 pairs, 128, DC, 392]
    imgTP = np.ascontiguousarray(imgTP.astype(f8))

    miscb = np.zeros((128, 48), np.float32)
    miscb[0, 0] = 1.0
    miscb[:, 1 : 1 + AC] = f32a(inputs["Wp1"]).reshape(AC, 128).T
    miscb[:, 5 : 5 + AC] = f32a(inputs["Wp2"]).reshape(AC, 128).T
    miscb[0, 9 : 9 + B] = 1.0
    miscb = np.ascontiguousarray(miscb.astype(bf))

    # QP1 = ques @ W_qa1 + b_qa1, computed on host in fp32
    qp1_full = ques @ f32a(inputs["W_qa1"]) + f32a(inputs["b_qa1"])  # [256, 512]

    shared = {
        "wia1": wchunk(inputs["W_ia1"], f8),
        "wia2": wchunk(inputs["W_ia2"], f8),
        "wqa2": wchunk(inputs["W_qa2"]),
        "wfc": wchunk(inputs["W_fc"]),
        "bfc": np.ascontiguousarray(f32a(inputs["b_fc"]).reshape(1, O).astype(bf)),
        "miscb": miscb,
    }
    bqa2T = acolT(inputs["b_qa2"], np.float32)
    in_maps = []
    for core in range(N_CORES):
        sl = slice(core * B, (core + 1) * B)
        slp = slice(core * NPAIR, (core + 1) * NPAIR)
        m = dict(shared)
        m["imgN"] = imgN[sl]
        m["imgTP"] = imgTP[slp]
        m["quesN"] = np.ascontiguousarray(ques[sl])
        miscf = np.zeros((128, 140), np.float32)
        miscf[0:8, 0:8] = np.eye(8)
        miscf[:, 8 : 8 + AC] = bqa2T
        miscf[:, 12 : 12 + AC * B] = (
            qp1_full[sl].reshape(B, AC, 128).transpose(2, 1, 0).reshape(128, AC * B)
        )
        m["miscf"] = np.ascontiguousarray(miscf)
        in_maps.append(m)
    return in_maps


def kernel_run(inputs, trace=False):
    from concourse.bass_utils import run_bass_kernel_spmd

    nc = _get_nc()
    in_maps = _make_in_maps(inputs)
    res = run_bass_kernel_spmd(nc, in_maps, core_ids=list(range(N_CORES)), trace=trace)
    out = np.concatenate([r["score"] for r in res.results], axis=0)
    return out, res


def kernel(**inputs):
    out, _ = kernel_run(inputs)
    return out
